# revision 20
# baseline (speedup 1.0000x reference)
"""GNN message-passing kernel for 8 TRN2 NeuronCores (Bass/Tile).

Strategy (v2):
- Destination-sharded: core c owns node rows [c*NPC, (c+1)*NPC).
- Phase 1 (replicated): every core computes full x = MLP2(x_in):
  mm1 feature-major (512-col chunks), mm2 emitted ROW-major per 128-block
  (lhsT = h-block, rhs = W2) -> bf16 row-major x table in DRAM (single
  tensor, lo/hi halves addressed as views for int16 gather indices).
  No PE transposes, no f32 feature-major table.
- Edge phase: edges sorted by (dest-half of source, dest 128-block);
  batched dma_gather (4 dest-groups per instruction) to amortize the
  ~1us SWDGE fixed overhead; S blocks generated ON-CHIP per 128-edge
  block with one fused DVE tensor_scalar (is_equal x mult) from a tiny
  (dloc, val) stream; PSUM accumulation of gx^T @ S -> aggT.
- Phase 3: own-slice x reloaded feature-major via transposed DMA with a
  dynamic row offset; small feature-major matmuls + sigmoid/tanh gates.
"""
import numpy as np
import ml_dtypes
import concourse.bass as bass
import concourse.bacc as bacc
import concourse.tile as tile
from concourse import mybir
from concourse.bass import ds
from concourse.bass_utils import run_bass_kernel_spmd

BF16 = mybir.dt.bfloat16
F32 = mybir.dt.float32
I16 = mybir.dt.int16
AF = mybir.ActivationFunctionType
OP = mybir.AluOpType
P = 128

# ---------------------------------------------------------------- tile patch
def _install_tile_patch():
    """walrus in this container accepts only one sync-wait per instruction;
    split the final drain's waits onto separate SP nops."""
    from concourse.tile import ScopedClock

    def _drain_and_barrier(self, tick_clock, wait_clock):
        nc = self.nc
        tmp = nc.sync.nop(nofuse=True)
        wait_clock.add_sem_waits(tmp.ins, ScopedClock({None: tick_clock.global_clock}))
        si = tmp.ins.sync_info
        waits = list(si.on_wait) if (si is not None and si.on_wait) else []
        if len(waits) > 1:
            si.on_wait = waits[:1]
            for w in waits[1:]:
                n2 = nc.sync.nop(nofuse=True)
                n2.ins.sync_info = mybir.SyncInfo(on_wait=[w], on_update=[])
        nc.sync.drain()
        nc.all_engine_barrier()
        assert self.sems is not None
        popped = nc._tile_sem_poison_stack.pop()
        assert popped is self._sem_poison
        nc.clear_and_free_semaphores(list(self.sems.allocated().values()))
        nc.all_engine_barrier()

    tile.TileContext._drain_and_barrier = _drain_and_barrier

_WS_CTR = [0]

def _split_multi_waits(nc):
    """Hoist extra sync-waits onto standalone nops (1-wait-per-inst walrus)."""
    for f in nc.m.functions:
        for bb in f.blocks:
            out, changed = [], False
            for ins in bb.instructions:
                si = ins.sync_info
                waits = list(si.on_wait) if (si is not None and si.on_wait) else []
                if len(waits) > 1:
                    changed = True
                    for w in waits[:-1]:
                        _WS_CTR[0] += 1
                        nop = mybir.InstNoOp(name=f"WS-{_WS_CTR[0]}", ins=[], outs=[])
                        nop.engine = ins.engine
                        nop.sync_info = mybir.SyncInfo(on_wait=[w], on_update=[])
                        out.append(nop)
                    si.on_wait = waits[-1:]
                out.append(ins)
            if changed:
                bb.instructions = out

_install_tile_patch()

# ---------------------------------------------------------------- config
class Cfg:
    def __init__(self, N, E, ncores=8, chunk=512, nbatch=4):
        self.N = N
        self.E = E
        self.ncores = ncores
        self.NPAD = ((N + ncores * P - 1) // (ncores * P)) * (ncores * P)
        self.NPC = self.NPAD // ncores          # rows per core
        self.G = self.NPC // P                  # dest blocks per core
        self.HALF = self.NPAD // 2              # lo/hi split row
        self.chunk = chunk                      # phase-1 column chunk
        self.NB = nbatch                        # dest groups per gather batch
        self.D = P

# ------------------------------------------------------------ preprocessing
def preprocess(cfg, rows, cols, vals):
    """Sort/group edges; per-core idx + (dloc,val) streams, h-major layout."""
    nc_, G, NPC, HALF = cfg.ncores, cfg.G, cfg.NPC, cfg.HALF
    rows = np.asarray(rows, np.int64)
    cols = np.asarray(cols, np.int64)
    vals = np.asarray(vals, np.float32)

    half = (cols >= HALF).astype(np.int64)
    core_id = rows // NPC
    g_id = (rows % NPC) // P
    # h-major within core so each (h, batch-of-g) is contiguous
    key = (core_id * 2 + half) * G + g_id
    order = np.argsort(key, kind='stable')
    key_s = key[order]
    cols_s = cols[order]
    vals_s = vals[order]
    dloc_s = (rows[order] % P).astype(np.int64)

    ngroups = nc_ * 2 * G
    counts = np.bincount(key_s, minlength=ngroups).reshape(nc_, 2 * G)
    Cblk = np.maximum(1, -(-counts.max(axis=0) // P))     # [2G] uniform
    B = int(Cblk.sum())
    icols = 8 * B
    run_starts = np.concatenate([[0], np.cumsum(counts.reshape(-1))])

    idx_arr = np.zeros((nc_, P, icols), np.int16)
    s_arr = np.zeros((nc_, P, B * P), ml_dtypes.bfloat16)

    for c in range(nc_):
        icol = 0
        boff = 0
        for hg in range(2 * G):
            h = hg // G
            C = int(Cblk[hg])
            NI = C * P
            gk = c * 2 * G + hg
            s, e = run_starts[gk], run_starts[gk + 1]
            n = int(e - s)
            idxs = np.zeros(NI, np.int64)
            idxs[:n] = cols_s[s:e] - h * HALF
            blk = idxs.reshape(NI // 16, 16).T.astype(np.int16)
            idx_arr[c, :, icol:icol + NI // 16] = np.tile(blk, (8, 1))
            icol += NI // 16
            # S stream: S[j, d] = val_j * (dloc_j == d); block k edge j ->
            # partition j, cols [(boff+k)*P : (boff+k+1)*P]
            Sg = np.zeros((NI, P), np.float32)
            Sg[np.arange(n), dloc_s[s:e]] = vals_s[s:e]
            Sg = Sg.reshape(C, P, P).transpose(1, 0, 2).reshape(P, C * P)
            s_arr[c, :, boff * P:(boff + C) * P] = Sg.astype(ml_dtypes.bfloat16)
            boff += C
    return Cblk, idx_arr, s_arr

def make_batches(cfg, Cblk):
    """[(h, [g...], [C...], icol0, boff0)] with h-major contiguous streams."""
    G, NB = cfg.G, cfg.NB
    batches = []
    icol = 0
    boff = 0
    for h in range(2):
        for g0 in range(0, G, NB):
            gs = list(range(g0, min(g0 + NB, G)))
            Cs = [int(Cblk[h * G + g]) for g in gs]
            batches.append((h, gs, Cs, icol, boff))
            icol += 8 * sum(Cs)
            boff += sum(Cs)
    return batches

# ------------------------------------------------------------ device build
def build_nc(cfg, Cblk, bias_zero, split=True):
    nc_, G, NPC, NPAD, HALF, CH = cfg.ncores, cfg.G, cfg.NPC, cfg.NPAD, cfg.HALF, cfg.chunk
    B = int(Cblk.sum())
    icols = 8 * B
    n_ch = NPAD // CH             # phase-1 chunks
    sub = CH // P                 # 128-blocks per chunk
    batches = make_batches(cfg, Cblk)
    SCmax = max(sum(Cs) for _, _, Cs, _, _ in batches)

    nc = bacc.Bacc("TRN2", target_bir_lowering=False, debug=False,
                   num_devices=nc_, num_swdge_queues=4,
                   dynamic_dma_scratch_size=int(__import__('os').environ.get(
                       'K_SCRATCH', '32768')))

    x_inT = nc.dram_tensor("x_inT", [P, NPAD], BF16, kind="ExternalInput")
    idx_in = nc.dram_tensor("idx", [P, icols], I16, kind="ExternalInput")
    s_in = nc.dram_tensor("sstream", [P, B * P], BF16, kind="ExternalInput")
    w_in = nc.dram_tensor("wts", [P, 10 * P], BF16, kind="ExternalInput")
    b_in = nc.dram_tensor("bias", [P, 7], F32, kind="ExternalInput")
    ident_in = nc.dram_tensor("ident", [P, P], F32, kind="ExternalInput")
    if not bias_zero:
        b2r_in = nc.dram_tensor("b2row", [P, P], F32, kind="ExternalInput")
    out_d = nc.dram_tensor("out", [P, NPC], F32, kind="ExternalOutput")

    x_lo = nc.dram_tensor("x_lo", [HALF, P], BF16)
    x_hi = nc.dram_tensor("x_hi", [NPAD - HALF, P], BF16)

    with tile.TileContext(nc) as tc:
        with tc.tile_pool(name="const", bufs=1) as cp, \
             tc.tile_pool(name="p1", bufs=3) as p1, \
             tc.tile_pool(name="gx", bufs=3) as gxp, \
             tc.tile_pool(name="sm", bufs=2) as smp, \
             tc.tile_pool(name="ix", bufs=4) as ixp, \
             tc.tile_pool(name="p3", bufs=2) as p3, \
             tc.tile_pool(name="psA", bufs=2, space="PSUM") as psA, \
             tc.tile_pool(name="psB", bufs=2, space="PSUM") as psB, \
             tc.tile_pool(name="psG", bufs=2, space="PSUM") as psG:

            wt = cp.tile([P, 10, P], BF16, name="wt")
            nc.sync.dma_start(wt[:], w_in[:].rearrange("p (k f) -> p k f", k=10))
            bt = cp.tile([P, 7], F32, name="bt")
            nc.sync.dma_start(bt[:], b_in[:])
            idt = cp.tile([P, P], F32, name="idt")
            nc.sync.dma_start(idt[:], ident_in[:])
            idtb = cp.tile([P, P], BF16, name="idtb")
            nc.vector.tensor_copy(idtb[:], idt[:])
            if not bias_zero:
                b2r = cp.tile([P, P], F32, name="b2r")
                nc.sync.dma_start(b2r[:], b2r_in[:])
            aggT = cp.tile([P, NPC], BF16, name="aggT")

            # ---------------- phase 1: x = MLP2(x_in), replicated ----------
            for rc in range(n_ch):
                xin_c = p1.tile([P, CH], BF16, tag="xin")
                nc.sync.dma_start(xin_c[:], x_inT[:, rc * CH:(rc + 1) * CH])
                ps1 = psA.tile([P, CH], F32, tag="psA")
                nc.tensor.matmul(ps1[:], lhsT=wt[:, 0, :], rhs=xin_c[:],
                                 start=True, stop=True)
                h_c = p1.tile([P, CH], BF16, tag="h")
                nc.scalar.activation(h_c[:], ps1[:], AF.Relu, bias=bt[:, 0:1])
                # mm2 emitted row-major: out[n, f] = sum_fh h[fh, n] W2[fh, f]
                ps2 = psB.tile([P, sub, P], F32, tag="psB")
                for j in range(sub):
                    nc.tensor.matmul(ps2[:, j, :], lhsT=h_c[:, j * P:(j + 1) * P],
                                     rhs=wt[:, 1, :], start=True, stop=True)
                xrow = p1.tile([P, sub, P], BF16, tag="xrow")
                if bias_zero:
                    nc.vector.tensor_copy(
                        xrow[:].rearrange("p a b -> p (a b)"),
                        ps2[:].rearrange("p a b -> p (a b)"))
                else:
                    for j in range(sub):
                        nc.vector.tensor_tensor(xrow[:, j, :], ps2[:, j, :],
                                                b2r[:], OP.add)
                base = rc * CH
                if base < HALF:
                    dst = x_lo[base:base + CH, :]
                else:
                    dst = x_hi[base - HALF:base - HALF + CH, :]
                nc.sync.dma_start(dst.rearrange("(k p) f -> p k f", p=P), xrow[:])

            # ---------------- edge phase: batched gather + on-chip S -------
            for bi, (h, gs, Cs, icol0, boff0) in enumerate(batches):
                SC = sum(Cs)
                NI = SC * P
                ixt = ixp.tile([P, SCmax * 8], I16, tag="ix")
                nc.sync.dma_start(ixt[:, :SC * 8], idx_in[:, icol0:icol0 + SC * 8])
                gx = gxp.tile([P, SCmax, P], BF16, tag="gx")
                src_t = x_lo if h == 0 else x_hi
                gmax = int(__import__('os').environ.get('K_GMAX', '100000')) // P
                for k0 in range(0, SC, gmax):
                    kc = min(gmax, SC - k0)
                    nc.gpsimd.dma_gather(gx[:, k0:k0 + kc, :], src_t[:],
                                         ixt[:, k0 * 8:(k0 + kc) * 8],
                                         kc * P, kc * P, P, single_packet=True,
                                         queue_num=(bi + k0 // gmax) % 4)
                s_t = smp.tile([P, SCmax, P], BF16, tag="s")
                nc.scalar.dma_start(s_t[:, :SC, :].rearrange("p a b -> p (a b)"),
                                    s_in[:, boff0 * P:(boff0 + SC) * P])
                kb = 0
                for gi, g in enumerate(gs):
                    C = Cs[gi]
                    psg = psG.tile([P, P], F32, tag="psG")
                    for k in range(C):
                        nc.tensor.matmul(psg[:], lhsT=gx[:, kb, :],
                                         rhs=s_t[:, kb, :],
                                         start=(k == 0), stop=(k == C - 1))
                        kb += 1
                    if h == 0:
                        nc.scalar.activation(aggT[:, g * P:(g + 1) * P], psg[:], AF.Copy)
                    else:
                        nc.vector.tensor_tensor(aggT[:, g * P:(g + 1) * P],
                                                aggT[:, g * P:(g + 1) * P],
                                                psg[:], OP.add)

            # ---------------- phase 3: gates on owned rows -----------------
            pid = nc.sync.partition_id()
            pid_lo = nc.s_assert_within(pid, 0, nc_ // 2 - 1,
                                        skip_runtime_assert=True)
            pid_hi = nc.s_assert_within(pid, nc_ // 2, nc_ - 1,
                                        skip_runtime_assert=True)
            chunks = []
            off = 0
            while off < NPC:
                w = min(CH, NPC - off)
                chunks.append((off, w))
                off += w
            for (off, w) in chunks:
                nb = w // P
                # own-slice rows; the range lives in exactly one of
                # x_lo / x_hi depending on pid, so issue both with OOB-skip
                # and exactly one lands.
                xr3 = p3.tile([P, sub, P], BF16, tag="xr3")
                if int(__import__('os').environ.get('K_P3COND', '1')):
                    src_lo = x_lo[ds(pid_lo * NPC + off, w), :].rearrange(
                        "(k p) f -> p k f", p=P)
                    nc.sync.dma_start(xr3[:, :nb, :],
                                      nc.ap_or_oob(src_lo, pid < (nc_ // 2)),
                                      bounds_check="skip_entire_dma")
                    src_hi = x_hi[ds(pid_hi * NPC - HALF + off, w), :].rearrange(
                        "(k p) f -> p k f", p=P)
                    nc.sync.dma_start(xr3[:, :nb, :],
                                      nc.ap_or_oob(src_hi, pid >= (nc_ // 2)),
                                      bounds_check="skip_entire_dma")
                else:
                    # bisect mode: unconditional (numerically wrong for hi pids)
                    src_lo = x_lo[ds(pid_lo * NPC + off, w), :].rearrange(
                        "(k p) f -> p k f", p=P)
                    nc.sync.dma_start(xr3[:, :nb, :], src_lo)
                xc_b = p3.tile([P, CH], BF16, tag="xc_b")
                for t in range(nb):
                    psT = psG.tile([P, P], BF16, tag="psT")
                    nc.tensor.transpose(psT[:], xr3[:, t, :], idtb[:])
                    nc.vector.tensor_copy(xc_b[:, t * P:(t + 1) * P], psT[:])
                agg_c = aggT[:, off:off + w]
                psa = psA.tile([P, CH], F32, tag="psA")
                nc.tensor.matmul(psa[:, :w], lhsT=wt[:, 2, :], rhs=agg_c,
                                 start=True, stop=True)
                h1 = p3.tile([P, CH], BF16, tag="h1")
                nc.scalar.activation(h1[:, :w], psa[:, :w], AF.Relu, bias=bt[:, 2:3])
                psb = psB.tile([P, CH], F32, tag="psB")
                nc.tensor.matmul(psb[:, :w], lhsT=wt[:, 3, :], rhs=h1[:, :w],
                                 start=True, stop=True)
                o_c = p3.tile([P, CH], BF16, tag="o")
                nc.vector.tensor_scalar(o_c[:, :w], psb[:, :w], bt[:, 3:4], None, OP.add)
                # z
                psz = psA.tile([P, CH], F32, tag="psA")
                nc.tensor.matmul(psz[:, :w], lhsT=wt[:, 4, :], rhs=o_c[:, :w],
                                 start=True, stop=False)
                nc.tensor.matmul(psz[:, :w], lhsT=wt[:, 5, :], rhs=xc_b[:, :w],
                                 start=False, stop=True)
                z_c = p3.tile([P, CH], BF16, tag="z")
                nc.scalar.activation(z_c[:, :w], psz[:, :w], AF.Sigmoid, bias=bt[:, 4:5])
                # r
                psr = psB.tile([P, CH], F32, tag="psB")
                nc.tensor.matmul(psr[:, :w], lhsT=wt[:, 6, :], rhs=o_c[:, :w],
                                 start=True, stop=False)
                nc.tensor.matmul(psr[:, :w], lhsT=wt[:, 7, :], rhs=xc_b[:, :w],
                                 start=False, stop=True)
                r_c = p3.tile([P, CH], BF16, tag="r")
                nc.scalar.activation(r_c[:, :w], psr[:, :w], AF.Sigmoid, bias=bt[:, 5:6])
                rx = p3.tile([P, CH], BF16, tag="rx")
                nc.vector.tensor_tensor(rx[:, :w], r_c[:, :w], xc_b[:, :w], OP.mult)
                # h
                psh = psA.tile([P, CH], F32, tag="psA")
                nc.tensor.matmul(psh[:, :w], lhsT=wt[:, 8, :], rhs=o_c[:, :w],
                                 start=True, stop=False)
                nc.tensor.matmul(psh[:, :w], lhsT=wt[:, 9, :], rhs=rx[:, :w],
                                 start=False, stop=True)
                hh = p3.tile([P, CH], BF16, tag="hh")
                nc.scalar.activation(hh[:, :w], psh[:, :w], AF.Tanh, bias=bt[:, 6:7])
                # out = x + z*(h - x)
                hmx = p3.tile([P, CH], F32, tag="hmx")
                nc.vector.tensor_tensor(hmx[:, :w], hh[:, :w], xc_b[:, :w], OP.subtract)
                zd = p3.tile([P, CH], F32, tag="zd")
                nc.vector.tensor_tensor(zd[:, :w], z_c[:, :w], hmx[:, :w], OP.mult)
                oc = p3.tile([P, CH], F32, tag="oc")
                nc.vector.tensor_tensor(oc[:, :w], zd[:, :w], xc_b[:, :w], OP.add)
                nc.sync.dma_start(out_d[:, off:off + w], oc[:, :w])

    nc.compile()
    if split:
        _split_multi_waits(nc)
    return nc

# ------------------------------------------------------------ host wrapper
_CACHE = {}
LAST_EXEC_NS = None

def prepare_inputs(cfg, inputs):
    N, NPAD, nc_ = cfg.N, cfg.NPAD, cfg.ncores
    x_in = np.asarray(inputs["x_in"], np.float32)
    x_pad = np.zeros((NPAD, P), np.float32)
    x_pad[:N] = x_in
    x_inT = np.ascontiguousarray(x_pad.T).astype(ml_dtypes.bfloat16)

    Cblk, idx_arr, s_arr = preprocess(
        cfg, inputs["rows"], inputs["cols"], inputs["vals"])

    names = ['m1_W1', 'm1_W2', 'm2_W1', 'm2_W2', 'Wu1', 'Wu2', 'Wr1', 'Wr2', 'Wo1', 'Wo2']
    wts = np.concatenate([np.asarray(inputs[n], np.float32) for n in names],
                         axis=1).astype(ml_dtypes.bfloat16)      # [128, 1280]
    bias = np.stack([
        np.asarray(inputs['m1_b1'], np.float32),
        np.asarray(inputs['m1_b2'], np.float32),
        np.asarray(inputs['m2_b1'], np.float32),
        np.asarray(inputs['m2_b2'], np.float32),
        np.asarray(inputs['bu1'], np.float32) + np.asarray(inputs['bu2'], np.float32),
        np.asarray(inputs['br1'], np.float32) + np.asarray(inputs['br2'], np.float32),
        np.asarray(inputs['bo1'], np.float32) + np.asarray(inputs['bo2'], np.float32),
    ], axis=1)                                                   # [128, 7]
    ident = np.eye(P, dtype=np.float32)
    b2 = np.asarray(inputs['m1_b2'], np.float32)
    bias_zero = bool(np.all(b2 == 0.0))

    in_maps = []
    for c in range(nc_):
        m = {
            "x_inT": x_inT, "idx": idx_arr[c], "sstream": s_arr[c],
            "wts": wts, "bias": bias, "ident": ident,
        }
        if not bias_zero:
            m["b2row"] = np.broadcast_to(b2, (P, P)).copy()
        in_maps.append(m)
    return Cblk, bias_zero, in_maps

def run(cfg, inputs, trace=False):
    global LAST_EXEC_NS
    Cblk, bias_zero, in_maps = prepare_inputs(cfg, inputs)
    key = (cfg.N, cfg.E, bias_zero, Cblk.tobytes())
    if key not in _CACHE:
        _CACHE[key] = build_nc(cfg, Cblk, bias_zero)
    nc = _CACHE[key]
    res = run_bass_kernel_spmd(nc, in_maps, core_ids=list(range(cfg.ncores)),
                               trace=trace)
    LAST_EXEC_NS = res.exec_time_ns
    outs = [res.results[c]["out"] for c in range(cfg.ncores)]   # [128, NPC] each
    full = np.concatenate([o.T for o in outs], axis=0)[:cfg.N]
    return np.ascontiguousarray(full, dtype=np.float32)


# ================================================================ entry point
_CFG = Cfg(50000, 1600000, ncores=8, chunk=512,
           nbatch=int(__import__('os').environ.get('K_NBATCH', '4')))

def kernel(**inputs):
    """Full-input GNN message-passing kernel on 8 TRN2 NeuronCores."""
    return run(_CFG, inputs, trace=False)


# revision 21
# speedup vs baseline: 1.0500x; 1.0500x over previous
"""GNN message-passing kernel for 8 TRN2 NeuronCores (Bass/Tile).

Strategy (v2):
- Destination-sharded: core c owns node rows [c*NPC, (c+1)*NPC).
- Phase 1 (replicated): every core computes full x = MLP2(x_in):
  mm1 feature-major (512-col chunks), mm2 emitted ROW-major per 128-block
  (lhsT = h-block, rhs = W2) -> bf16 row-major x table in DRAM (single
  tensor, lo/hi halves addressed as views for int16 gather indices).
  No PE transposes, no f32 feature-major table.
- Edge phase: edges sorted by (dest-half of source, dest 128-block);
  batched dma_gather (4 dest-groups per instruction) to amortize the
  ~1us SWDGE fixed overhead; S blocks generated ON-CHIP per 128-edge
  block with one fused DVE tensor_scalar (is_equal x mult) from a tiny
  (dloc, val) stream; PSUM accumulation of gx^T @ S -> aggT.
- Phase 3: own-slice x reloaded feature-major via transposed DMA with a
  dynamic row offset; small feature-major matmuls + sigmoid/tanh gates.
"""
import numpy as np
import ml_dtypes
import concourse.bass as bass
import concourse.bacc as bacc
import concourse.tile as tile
from concourse import mybir
from concourse.bass import ds
from concourse.bass_utils import run_bass_kernel_spmd

BF16 = mybir.dt.bfloat16
F32 = mybir.dt.float32
FP8 = mybir.dt.float8e4
I16 = mybir.dt.int16
AF = mybir.ActivationFunctionType
OP = mybir.AluOpType
P = 128

# ---------------------------------------------------------------- tile patch
def _install_tile_patch():
    """walrus in this container accepts only one sync-wait per instruction;
    split the final drain's waits onto separate SP nops."""
    from concourse.tile import ScopedClock

    def _drain_and_barrier(self, tick_clock, wait_clock):
        nc = self.nc
        tmp = nc.sync.nop(nofuse=True)
        wait_clock.add_sem_waits(tmp.ins, ScopedClock({None: tick_clock.global_clock}))
        si = tmp.ins.sync_info
        waits = list(si.on_wait) if (si is not None and si.on_wait) else []
        if len(waits) > 1:
            si.on_wait = waits[:1]
            for w in waits[1:]:
                n2 = nc.sync.nop(nofuse=True)
                n2.ins.sync_info = mybir.SyncInfo(on_wait=[w], on_update=[])
        nc.sync.drain()
        nc.all_engine_barrier()
        assert self.sems is not None
        popped = nc._tile_sem_poison_stack.pop()
        assert popped is self._sem_poison
        nc.clear_and_free_semaphores(list(self.sems.allocated().values()))
        nc.all_engine_barrier()

    tile.TileContext._drain_and_barrier = _drain_and_barrier

_WS_CTR = [0]

def _split_multi_waits(nc):
    """Hoist extra sync-waits onto standalone nops (1-wait-per-inst walrus)."""
    for f in nc.m.functions:
        for bb in f.blocks:
            out, changed = [], False
            for ins in bb.instructions:
                si = ins.sync_info
                waits = list(si.on_wait) if (si is not None and si.on_wait) else []
                if len(waits) > 1:
                    changed = True
                    for w in waits[:-1]:
                        _WS_CTR[0] += 1
                        nop = mybir.InstNoOp(name=f"WS-{_WS_CTR[0]}", ins=[], outs=[])
                        nop.engine = ins.engine
                        nop.sync_info = mybir.SyncInfo(on_wait=[w], on_update=[])
                        out.append(nop)
                    si.on_wait = waits[-1:]
                out.append(ins)
            if changed:
                bb.instructions = out

_install_tile_patch()

# ---------------------------------------------------------------- config
class Cfg:
    def __init__(self, N, E, ncores=8, chunk=512, nbatch=4):
        self.N = N
        self.E = E
        self.ncores = ncores
        self.NPAD = ((N + ncores * P - 1) // (ncores * P)) * (ncores * P)
        self.NPC = self.NPAD // ncores          # rows per core
        self.G = self.NPC // P                  # dest blocks per core
        self.HALF = self.NPAD // 2              # lo/hi split row
        self.chunk = chunk                      # phase-1 column chunk
        self.NB = nbatch                        # dest groups per gather batch
        self.D = P

# ------------------------------------------------------------ preprocessing
def preprocess(cfg, rows, cols, vals):
    """Sort/group edges; per-core idx + (dloc,val) streams, h-major layout."""
    nc_, G, NPC, HALF = cfg.ncores, cfg.G, cfg.NPC, cfg.HALF
    rows = np.asarray(rows, np.int64)
    cols = np.asarray(cols, np.int64)
    vals = np.asarray(vals, np.float32)

    half = (cols >= HALF).astype(np.int64)
    core_id = rows // NPC
    g_id = (rows % NPC) // P
    # h-major within core so each (h, batch-of-g) is contiguous
    key = (core_id * 2 + half) * G + g_id
    order = np.argsort(key, kind='stable')
    key_s = key[order]
    cols_s = cols[order]
    vals_s = vals[order]
    dloc_s = (rows[order] % P).astype(np.int64)

    ngroups = nc_ * 2 * G
    counts = np.bincount(key_s, minlength=ngroups).reshape(nc_, 2 * G)
    Cblk = np.maximum(1, -(-counts.max(axis=0) // P))     # [2G] uniform
    B = int(Cblk.sum())
    icols = 8 * B
    run_starts = np.concatenate([[0], np.cumsum(counts.reshape(-1))])

    idx_arr = np.zeros((nc_, P, icols), np.int16)
    s_arr = np.zeros((nc_, P, B * P), ml_dtypes.float8_e4m3)

    for c in range(nc_):
        icol = 0
        boff = 0
        for hg in range(2 * G):
            h = hg // G
            C = int(Cblk[hg])
            NI = C * P
            gk = c * 2 * G + hg
            s, e = run_starts[gk], run_starts[gk + 1]
            n = int(e - s)
            idxs = np.zeros(NI, np.int64)
            idxs[:n] = cols_s[s:e] - h * HALF
            blk = idxs.reshape(NI // 16, 16).T.astype(np.int16)
            idx_arr[c, :, icol:icol + NI // 16] = np.tile(blk, (8, 1))
            icol += NI // 16
            # S stream: S[j, d] = val_j * (dloc_j == d); block k edge j ->
            # partition j, cols [(boff+k)*P : (boff+k+1)*P]
            Sg = np.zeros((NI, P), np.float32)
            Sg[np.arange(n), dloc_s[s:e]] = vals_s[s:e]
            Sg = Sg.reshape(C, P, P).transpose(1, 0, 2).reshape(P, C * P)
            s_arr[c, :, boff * P:(boff + C) * P] = Sg.astype(ml_dtypes.float8_e4m3)
            boff += C
    return Cblk, idx_arr, s_arr

def make_batches(cfg, Cblk):
    """[(h, [g...], [C...], icol0, boff0)] with h-major contiguous streams."""
    G, NB = cfg.G, cfg.NB
    batches = []
    icol = 0
    boff = 0
    for h in range(2):
        for g0 in range(0, G, NB):
            gs = list(range(g0, min(g0 + NB, G)))
            Cs = [int(Cblk[h * G + g]) for g in gs]
            batches.append((h, gs, Cs, icol, boff))
            icol += 8 * sum(Cs)
            boff += sum(Cs)
    return batches

# ------------------------------------------------------------ device build
def build_nc(cfg, Cblk, bias_zero, split=True):
    nc_, G, NPC, NPAD, HALF, CH = cfg.ncores, cfg.G, cfg.NPC, cfg.NPAD, cfg.HALF, cfg.chunk
    B = int(Cblk.sum())
    icols = 8 * B
    n_ch = NPAD // CH             # phase-1 chunks
    sub = CH // P                 # 128-blocks per chunk
    batches = make_batches(cfg, Cblk)
    SCmax = max(sum(Cs) for _, _, Cs, _, _ in batches)

    nc = bacc.Bacc("TRN2", target_bir_lowering=False, debug=False,
                   num_devices=nc_, num_swdge_queues=4,
                   dynamic_dma_scratch_size=int(__import__('os').environ.get(
                       'K_SCRATCH', '32768')))

    x_inT = nc.dram_tensor("x_inT", [P, NPAD], BF16, kind="ExternalInput")
    idx_in = nc.dram_tensor("idx", [P, icols], I16, kind="ExternalInput")
    s_in = nc.dram_tensor("sstream", [P, B * P], FP8, kind="ExternalInput")
    w_in = nc.dram_tensor("wts", [P, 10 * P], BF16, kind="ExternalInput")
    b_in = nc.dram_tensor("bias", [P, 7], F32, kind="ExternalInput")
    ident_in = nc.dram_tensor("ident", [P, P], F32, kind="ExternalInput")
    if not bias_zero:
        b2r_in = nc.dram_tensor("b2row", [P, P], F32, kind="ExternalInput")
    out_d = nc.dram_tensor("out", [P, NPC], F32, kind="ExternalOutput")

    x_lo = nc.dram_tensor("x_lo", [HALF, P], BF16)
    x_hi = nc.dram_tensor("x_hi", [NPAD - HALF, P], BF16)

    with tile.TileContext(nc) as tc:
        with tc.tile_pool(name="const", bufs=1) as cp, \
             tc.tile_pool(name="p1", bufs=3) as p1, \
             tc.tile_pool(name="gx", bufs=3) as gxp, \
             tc.tile_pool(name="sm", bufs=2) as smp, \
             tc.tile_pool(name="ix", bufs=4) as ixp, \
             tc.tile_pool(name="p3", bufs=2) as p3, \
             tc.tile_pool(name="psA", bufs=2, space="PSUM") as psA, \
             tc.tile_pool(name="psB", bufs=2, space="PSUM") as psB, \
             tc.tile_pool(name="psG", bufs=2, space="PSUM") as psG:

            wt = cp.tile([P, 10, P], BF16, name="wt")
            nc.sync.dma_start(wt[:], w_in[:].rearrange("p (k f) -> p k f", k=10))
            bt = cp.tile([P, 7], F32, name="bt")
            nc.sync.dma_start(bt[:], b_in[:])
            idt = cp.tile([P, P], F32, name="idt")
            nc.sync.dma_start(idt[:], ident_in[:])
            idtb = cp.tile([P, P], BF16, name="idtb")
            nc.vector.tensor_copy(idtb[:], idt[:])
            if not bias_zero:
                b2r = cp.tile([P, P], F32, name="b2r")
                nc.sync.dma_start(b2r[:], b2r_in[:])
            aggT = cp.tile([P, NPC], BF16, name="aggT")

            # ---------------- phase 1: x = MLP2(x_in), replicated ----------
            for rc in range(n_ch):
                xin_c = p1.tile([P, CH], BF16, tag="xin")
                nc.sync.dma_start(xin_c[:], x_inT[:, rc * CH:(rc + 1) * CH])
                ps1 = psA.tile([P, CH], F32, tag="psA")
                nc.tensor.matmul(ps1[:], lhsT=wt[:, 0, :], rhs=xin_c[:],
                                 start=True, stop=True)
                h_c = p1.tile([P, CH], BF16, tag="h")
                nc.scalar.activation(h_c[:], ps1[:], AF.Relu, bias=bt[:, 0:1])
                # mm2 emitted row-major: out[n, f] = sum_fh h[fh, n] W2[fh, f]
                ps2 = psB.tile([P, sub, P], F32, tag="psB")
                for j in range(sub):
                    nc.tensor.matmul(ps2[:, j, :], lhsT=h_c[:, j * P:(j + 1) * P],
                                     rhs=wt[:, 1, :], start=True, stop=True)
                xrow = p1.tile([P, sub, P], BF16, tag="xrow")
                if bias_zero:
                    nc.vector.tensor_copy(
                        xrow[:].rearrange("p a b -> p (a b)"),
                        ps2[:].rearrange("p a b -> p (a b)"))
                else:
                    for j in range(sub):
                        nc.vector.tensor_tensor(xrow[:, j, :], ps2[:, j, :],
                                                b2r[:], OP.add)
                base = rc * CH
                if base < HALF:
                    dst = x_lo[base:base + CH, :]
                else:
                    dst = x_hi[base - HALF:base - HALF + CH, :]
                nc.sync.dma_start(dst.rearrange("(k p) f -> p k f", p=P), xrow[:])

            # ---------------- edge phase: batched gather + on-chip S -------
            for bi, (h, gs, Cs, icol0, boff0) in enumerate(batches):
                SC = sum(Cs)
                NI = SC * P
                ixt = ixp.tile([P, SCmax * 8], I16, tag="ix")
                nc.sync.dma_start(ixt[:, :SC * 8], idx_in[:, icol0:icol0 + SC * 8])
                gx = gxp.tile([P, SCmax, P], BF16, tag="gx")
                src_t = x_lo if h == 0 else x_hi
                gmax = int(__import__('os').environ.get('K_GMAX', '100000')) // P
                for k0 in range(0, SC, gmax):
                    kc = min(gmax, SC - k0)
                    nc.gpsimd.dma_gather(gx[:, k0:k0 + kc, :], src_t[:],
                                         ixt[:, k0 * 8:(k0 + kc) * 8],
                                         kc * P, kc * P, P, single_packet=True,
                                         queue_num=(bi + k0 // gmax) % 4)
                s_t = smp.tile([P, SCmax, P], FP8, tag="s")
                nc.scalar.dma_start(s_t[:, :SC, :].rearrange("p a b -> p (a b)"),
                                    s_in[:, boff0 * P:(boff0 + SC) * P])
                kb = 0
                for gi, g in enumerate(gs):
                    C = Cs[gi]
                    psg = psG.tile([P, P], F32, tag="psG")
                    for k in range(C):
                        nc.tensor.matmul(psg[:], lhsT=gx[:, kb, :],
                                         rhs=s_t[:, kb, :],
                                         start=(k == 0), stop=(k == C - 1))
                        kb += 1
                    if h == 0:
                        nc.scalar.activation(aggT[:, g * P:(g + 1) * P], psg[:], AF.Copy)
                    else:
                        nc.vector.tensor_tensor(aggT[:, g * P:(g + 1) * P],
                                                aggT[:, g * P:(g + 1) * P],
                                                psg[:], OP.add)

            # ---------------- phase 3: gates on owned rows -----------------
            pid = nc.sync.partition_id()
            pid_lo = nc.s_assert_within(pid, 0, nc_ // 2 - 1,
                                        skip_runtime_assert=True)
            pid_hi = nc.s_assert_within(pid, nc_ // 2, nc_ - 1,
                                        skip_runtime_assert=True)
            chunks = []
            off = 0
            while off < NPC:
                w = min(CH, NPC - off)
                chunks.append((off, w))
                off += w
            for (off, w) in chunks:
                nb = w // P
                # own-slice rows; the range lives in exactly one of
                # x_lo / x_hi depending on pid, so issue both with OOB-skip
                # and exactly one lands.
                xr3 = p3.tile([P, sub, P], BF16, tag="xr3")
                if int(__import__('os').environ.get('K_P3COND', '1')):
                    src_lo = x_lo[ds(pid_lo * NPC + off, w), :].rearrange(
                        "(k p) f -> p k f", p=P)
                    nc.sync.dma_start(xr3[:, :nb, :],
                                      nc.ap_or_oob(src_lo, pid < (nc_ // 2)),
                                      bounds_check="skip_entire_dma")
                    src_hi = x_hi[ds(pid_hi * NPC - HALF + off, w), :].rearrange(
                        "(k p) f -> p k f", p=P)
                    nc.sync.dma_start(xr3[:, :nb, :],
                                      nc.ap_or_oob(src_hi, pid >= (nc_ // 2)),
                                      bounds_check="skip_entire_dma")
                else:
                    # bisect mode: unconditional (numerically wrong for hi pids)
                    src_lo = x_lo[ds(pid_lo * NPC + off, w), :].rearrange(
                        "(k p) f -> p k f", p=P)
                    nc.sync.dma_start(xr3[:, :nb, :], src_lo)
                xc_b = p3.tile([P, CH], BF16, tag="xc_b")
                for t in range(nb):
                    psT = psG.tile([P, P], BF16, tag="psT")
                    nc.tensor.transpose(psT[:], xr3[:, t, :], idtb[:])
                    nc.vector.tensor_copy(xc_b[:, t * P:(t + 1) * P], psT[:])
                agg_c = aggT[:, off:off + w]
                psa = psA.tile([P, CH], F32, tag="psA")
                nc.tensor.matmul(psa[:, :w], lhsT=wt[:, 2, :], rhs=agg_c,
                                 start=True, stop=True)
                h1 = p3.tile([P, CH], BF16, tag="h1")
                nc.scalar.activation(h1[:, :w], psa[:, :w], AF.Relu, bias=bt[:, 2:3])
                psb = psB.tile([P, CH], F32, tag="psB")
                nc.tensor.matmul(psb[:, :w], lhsT=wt[:, 3, :], rhs=h1[:, :w],
                                 start=True, stop=True)
                o_c = p3.tile([P, CH], BF16, tag="o")
                nc.vector.tensor_scalar(o_c[:, :w], psb[:, :w], bt[:, 3:4], None, OP.add)
                # z
                psz = psA.tile([P, CH], F32, tag="psA")
                nc.tensor.matmul(psz[:, :w], lhsT=wt[:, 4, :], rhs=o_c[:, :w],
                                 start=True, stop=False)
                nc.tensor.matmul(psz[:, :w], lhsT=wt[:, 5, :], rhs=xc_b[:, :w],
                                 start=False, stop=True)
                z_c = p3.tile([P, CH], BF16, tag="z")
                nc.scalar.activation(z_c[:, :w], psz[:, :w], AF.Sigmoid, bias=bt[:, 4:5])
                # r
                psr = psB.tile([P, CH], F32, tag="psB")
                nc.tensor.matmul(psr[:, :w], lhsT=wt[:, 6, :], rhs=o_c[:, :w],
                                 start=True, stop=False)
                nc.tensor.matmul(psr[:, :w], lhsT=wt[:, 7, :], rhs=xc_b[:, :w],
                                 start=False, stop=True)
                r_c = p3.tile([P, CH], BF16, tag="r")
                nc.scalar.activation(r_c[:, :w], psr[:, :w], AF.Sigmoid, bias=bt[:, 5:6])
                rx = p3.tile([P, CH], BF16, tag="rx")
                nc.vector.tensor_tensor(rx[:, :w], r_c[:, :w], xc_b[:, :w], OP.mult)
                # h
                psh = psA.tile([P, CH], F32, tag="psA")
                nc.tensor.matmul(psh[:, :w], lhsT=wt[:, 8, :], rhs=o_c[:, :w],
                                 start=True, stop=False)
                nc.tensor.matmul(psh[:, :w], lhsT=wt[:, 9, :], rhs=rx[:, :w],
                                 start=False, stop=True)
                hh = p3.tile([P, CH], BF16, tag="hh")
                nc.scalar.activation(hh[:, :w], psh[:, :w], AF.Tanh, bias=bt[:, 6:7])
                # out = x + z*(h - x)
                hmx = p3.tile([P, CH], F32, tag="hmx")
                nc.vector.tensor_tensor(hmx[:, :w], hh[:, :w], xc_b[:, :w], OP.subtract)
                zd = p3.tile([P, CH], F32, tag="zd")
                nc.vector.tensor_tensor(zd[:, :w], z_c[:, :w], hmx[:, :w], OP.mult)
                oc = p3.tile([P, CH], F32, tag="oc")
                nc.vector.tensor_tensor(oc[:, :w], zd[:, :w], xc_b[:, :w], OP.add)
                nc.sync.dma_start(out_d[:, off:off + w], oc[:, :w])

    nc.compile()
    if split:
        _split_multi_waits(nc)
    return nc

# ------------------------------------------------------------ host wrapper
_CACHE = {}
LAST_EXEC_NS = None

def prepare_inputs(cfg, inputs):
    N, NPAD, nc_ = cfg.N, cfg.NPAD, cfg.ncores
    x_in = np.asarray(inputs["x_in"], np.float32)
    x_pad = np.zeros((NPAD, P), np.float32)
    x_pad[:N] = x_in
    x_inT = np.ascontiguousarray(x_pad.T).astype(ml_dtypes.bfloat16)

    Cblk, idx_arr, s_arr = preprocess(
        cfg, inputs["rows"], inputs["cols"], inputs["vals"])

    names = ['m1_W1', 'm1_W2', 'm2_W1', 'm2_W2', 'Wu1', 'Wu2', 'Wr1', 'Wr2', 'Wo1', 'Wo2']
    wts = np.concatenate([np.asarray(inputs[n], np.float32) for n in names],
                         axis=1).astype(ml_dtypes.bfloat16)      # [128, 1280]
    bias = np.stack([
        np.asarray(inputs['m1_b1'], np.float32),
        np.asarray(inputs['m1_b2'], np.float32),
        np.asarray(inputs['m2_b1'], np.float32),
        np.asarray(inputs['m2_b2'], np.float32),
        np.asarray(inputs['bu1'], np.float32) + np.asarray(inputs['bu2'], np.float32),
        np.asarray(inputs['br1'], np.float32) + np.asarray(inputs['br2'], np.float32),
        np.asarray(inputs['bo1'], np.float32) + np.asarray(inputs['bo2'], np.float32),
    ], axis=1)                                                   # [128, 7]
    ident = np.eye(P, dtype=np.float32)
    b2 = np.asarray(inputs['m1_b2'], np.float32)
    bias_zero = bool(np.all(b2 == 0.0))

    in_maps = []
    for c in range(nc_):
        m = {
            "x_inT": x_inT, "idx": idx_arr[c], "sstream": s_arr[c],
            "wts": wts, "bias": bias, "ident": ident,
        }
        if not bias_zero:
            m["b2row"] = np.broadcast_to(b2, (P, P)).copy()
        in_maps.append(m)
    return Cblk, bias_zero, in_maps

def run(cfg, inputs, trace=False):
    global LAST_EXEC_NS
    Cblk, bias_zero, in_maps = prepare_inputs(cfg, inputs)
    key = (cfg.N, cfg.E, bias_zero, Cblk.tobytes())
    if key not in _CACHE:
        _CACHE[key] = build_nc(cfg, Cblk, bias_zero)
    nc = _CACHE[key]
    res = run_bass_kernel_spmd(nc, in_maps, core_ids=list(range(cfg.ncores)),
                               trace=trace)
    LAST_EXEC_NS = res.exec_time_ns
    outs = [res.results[c]["out"] for c in range(cfg.ncores)]   # [128, NPC] each
    full = np.concatenate([o.T for o in outs], axis=0)[:cfg.N]
    return np.ascontiguousarray(full, dtype=np.float32)


# ================================================================ entry point
_CFG = Cfg(50000, 1600000, ncores=8, chunk=512,
           nbatch=int(__import__('os').environ.get('K_NBATCH', '4')))

def kernel(**inputs):
    """Full-input GNN message-passing kernel on 8 TRN2 NeuronCores."""
    return run(_CFG, inputs, trace=False)
